# revision 10
# baseline (speedup 1.0000x reference)
"""Trainium2 Bass kernel for nn_Block_12154757448460 (spiking retention transformer).

v3: chunked linear retention (256-wide state chunks), row-sharded k via the
v AllToAll (no PE transposes), bf16 LIF states, Sign-spike LIF on ACT for
r/p/h/m paths, engine-queue separation (sync=data DMAs, gpsimd=globals+
collectives, scalar=ACT only).

Sharding over 8 NeuronCores (1 chip):
  - q,k projections: column-sharded by head for the diagonal blocks; k is
    ALSO row-sharded through stage B so the AllToAll delivers k in [m, d]
    layout for the KV state updates (no transposes).
  - v (+k) projection, output projection, MLP: row-sharded.
  - attention: per-head chunked bidirectional retention: 128x128 masked
    diagonal blocks + 64x64 fwd/bwd decay states with 256-element hops.
  - LIF spikes on r/p/h/m paths emitted as +-1 via ACT Sign; affine fix
    folded into next layer's bias via rowsum(W). q/k/v spikes stay {0,1}.
  - BatchNorm folded into weights/biases on host; LIF in scaled-threshold
    form with bf16 states (pt terms are bf16 anyway).
"""
import os
import sys
import numpy as np
import ml_dtypes

for _p in ("/root/.axon_site/_ro/trn_rl_repo", "/opt/trn_rl_repo"):
    if os.path.isdir(_p) and _p not in sys.path:
        sys.path.append(_p)

bf16 = ml_dtypes.bfloat16

T, B, N, C = 4, 4, 1024, 512
H, D = 8, 64
HID = 4 * C
NCORES = 8
NL = N // NCORES          # 128
RR = T * B * NL           # 2048 rows per core
RF = T * B * N            # 16384 full rows
NI = T * B                # 16 attention instances
LS = 256                  # state chunk
NC4 = N // LS             # 4 state chunks
EPS = 1e-5
SCALE = D ** -0.5

LAST_EXEC_NS = None
_CACHED = None


def _fold_bn(W, bias, g, beta, rm, rv):
    ghat = (np.asarray(g, np.float64) / np.sqrt(np.asarray(rv, np.float64) + EPS))
    Wf = (np.asarray(W, np.float64) * ghat[None, :]).astype(np.float32)
    bf_ = ((np.asarray(bias, np.float64) - np.asarray(rm, np.float64)) * ghat
           + np.asarray(beta, np.float64)).astype(np.float32)
    return Wf, bf_


def _build_nc():
    import concourse.bass as bass  # noqa: F401
    import concourse.bacc as bacc
    import concourse.mybir as mybir
    from concourse import tile

    f32 = mybir.dt.float32
    b16 = mybir.dt.bfloat16
    ALU = mybir.AluOpType
    ACT = mybir.ActivationFunctionType

    nc = bacc.Bacc("TRN2", target_bir_lowering=False, debug=False,
                   num_devices=NCORES)

    # ---- external inputs (per-core values via in_maps) ----
    xT = nc.declare_dram_parameter("xT", [C, RF], b16, isOutput=False)
    xrT = nc.declare_dram_parameter("xrT", [C, RR], b16, isOutput=False)
    xp1 = nc.declare_dram_parameter("xp1", [C, RR], b16, isOutput=False)
    wqk = nc.declare_dram_parameter("wqk", [C, 128], b16, isOutput=False)
    bqk = nc.declare_dram_parameter("bqk", [128, T], f32, isOutput=False)
    wv = nc.declare_dram_parameter("wv", [C, C], b16, isOutput=False)
    bv4 = nc.declare_dram_parameter("bv4", [128, 4 * C], b16, isOutput=False)
    wk = nc.declare_dram_parameter("wk", [C, C], b16, isOutput=False)
    bk4 = nc.declare_dram_parameter("bk4", [128, 4 * C], b16, isOutput=False)
    wp = nc.declare_dram_parameter("wp", [C, C], b16, isOutput=False)
    bpn = nc.declare_dram_parameter("bpn", [128, 4 * T], f32, isOutput=False)
    w1 = nc.declare_dram_parameter("w1", [C, HID], b16, isOutput=False)
    b1n = nc.declare_dram_parameter("b1n", [128, 16 * T], f32, isOutput=False)
    w2 = nc.declare_dram_parameter("w2", [HID, C], b16, isOutput=False)
    b2n = nc.declare_dram_parameter("b2n", [128, 4 * T], f32, isOutput=False)
    gsc = nc.declare_dram_parameter("gsc", [64, 4 * N], b16, isOutput=False)
    vscf = nc.declare_dram_parameter("vscf", [128, 8 * 128], b16, isOutput=False)
    vscb = nc.declare_dram_parameter("vscb", [128, 8 * 128], b16, isOutput=False)
    gLt = nc.declare_dram_parameter("gLt", [128, 1], f32, isOutput=False)
    thn = nc.declare_dram_parameter("thn", [128, 2 * T], f32, isOutput=False)
    msku = nc.declare_dram_parameter("msku", [128, 128], mybir.dt.uint8,
                                     isOutput=False)
    out_e = nc.declare_dram_parameter("out", [C, RR], f32, isOutput=True)

    # ---- internal DRAM ----
    vc_in = nc.dram_tensor("vc_in", [NCORES, RR, 2 * D], b16)
    vc_out = nc.dram_tensor("vc_out", [NCORES, RR, 2 * D], b16)
    rc_in_a = nc.dram_tensor("rc_in_a", [NCORES, D, RR // 2], b16)
    rc_out_a = nc.dram_tensor("rc_out_a", [NCORES, D, RR // 2], b16)
    rc_in_b = nc.dram_tensor("rc_in_b", [NCORES, D, RR // 2], b16)
    rc_out_b = nc.dram_tensor("rc_out_b", [NCORES, D, RR // 2], b16)

    rg = [list(range(NCORES))]

    with tile.TileContext(nc) as tc:
      with (
          tc.tile_pool(name="glob", bufs=1) as GP,
          tc.tile_pool(name="work", bufs=3) as WP,
          tc.tile_pool(name="spike", bufs=2) as SP,
          tc.tile_pool(name="xa", bufs=6) as XA,
      ):
        # ---------- stage-B-critical globals first (sync queue) ----------
        xrT_sb = GP.tile([128, 4 * RR], b16, tag="xrT_sb")
        for kk in range(4):
            nc.sync.dma_start(out=xrT_sb[:, kk * RR:(kk + 1) * RR],
                              in_=xrT[kk * 128:(kk + 1) * 128, :])
        wv_sb = GP.tile([128, 4 * C], b16, tag="wv_sb")
        for kk in range(4):
            nc.sync.dma_start(out=wv_sb[:, kk * C:(kk + 1) * C],
                              in_=wv[kk * 128:(kk + 1) * 128, :])
        wk_sb = GP.tile([128, 4 * C], b16, tag="wk_sb")
        for kk in range(4):
            nc.sync.dma_start(out=wk_sb[:, kk * C:(kk + 1) * C],
                              in_=wk[kk * 128:(kk + 1) * 128, :])
        bv4_sb = GP.tile([128, 4 * C], b16, tag="bv4_sb")
        nc.sync.dma_start(out=bv4_sb[:], in_=bv4[:])
        bk4_sb = GP.tile([128, 4 * C], b16, tag="bk4_sb")
        nc.sync.dma_start(out=bk4_sb[:], in_=bk4[:])
        # ---------- remaining globals (gpsimd queue) ----------
        wqk_sb = GP.tile([128, 4 * 128], b16, tag="wqk_sb")
        for kk in range(4):
            nc.gpsimd.dma_start(out=wqk_sb[:, kk * 128:(kk + 1) * 128],
                                in_=wqk[kk * 128:(kk + 1) * 128, :])
        bqk_sb = GP.tile([128, T], f32, tag="bqk_sb")
        nc.gpsimd.dma_start(out=bqk_sb[:], in_=bqk[:])
        gsc_sb = GP.tile([64, 4 * N], b16, tag="gsc_sb")
        nc.gpsimd.dma_start(out=gsc_sb[:], in_=gsc[:])
        vscf_sb = GP.tile([128, 8 * 128], b16, tag="vscf_sb")
        nc.gpsimd.dma_start(out=vscf_sb[:], in_=vscf[:])
        vscb_sb = GP.tile([128, 8 * 128], b16, tag="vscb_sb")
        nc.gpsimd.dma_start(out=vscb_sb[:], in_=vscb[:])
        gLt_sb = GP.tile([128, 1], f32, tag="gLt_sb")
        nc.gpsimd.dma_start(out=gLt_sb[:], in_=gLt[:])
        thn_sb = GP.tile([128, 2 * T], f32, tag="thn_sb")
        nc.gpsimd.dma_start(out=thn_sb[:], in_=thn[:])
        msku_sb = GP.tile([128, 128], mybir.dt.uint8, tag="msku_sb")
        nc.gpsimd.dma_start(out=msku_sb[:], in_=msku[:])
        xp1_sb = GP.tile([128, 4 * RR], b16, tag="xp1_sb")
        for kk in range(4):
            nc.gpsimd.dma_start(out=xp1_sb[:, kk * RR:(kk + 1) * RR],
                                in_=xp1[kk * 128:(kk + 1) * 128, :])
        wp_sb = GP.tile([128, 4 * C], b16, tag="wp_sb")
        for kk in range(4):
            nc.gpsimd.dma_start(out=wp_sb[:, kk * C:(kk + 1) * C],
                                in_=wp[kk * 128:(kk + 1) * 128, :])
        bpn_sb = GP.tile([128, 4 * T], f32, tag="bpn_sb")
        nc.gpsimd.dma_start(out=bpn_sb[:], in_=bpn[:])
        b1n_sb = GP.tile([128, 16 * T], f32, tag="b1n_sb")
        nc.gpsimd.dma_start(out=b1n_sb[:], in_=b1n[:])
        b2n_sb = GP.tile([128, 4 * T], f32, tag="b2n_sb")
        nc.gpsimd.dma_start(out=b2n_sb[:], in_=b2n[:])
        # cross-phase result tiles
        x1p_all = GP.tile([128, 4 * RR], b16, tag="x1p_all")
        p_st = GP.tile([128, 4 * 512], b16, tag="p_st")

        # ================= phase 1 =================
        with tc.tile_pool(name="ph1", bufs=1) as P1:
            qk_sp = P1.tile([128, RF], b16, tag="qk_sp")   # [q_h; k_h] spikes
            rv_st = P1.tile([128, 2 * N], b16, tag="rv_st")  # 2 pair slots

            # ----- stage B: v,k projections (row-shard, row-major) -----
            with tc.tile_pool(name="psb", bufs=2, space="PSUM") as PSB, \
                 tc.tile_pool(name="stb", bufs=1) as STB:
                v_st = STB.tile([128, B * C], b16, tag="v_st")
                k_st = STB.tile([128, B * C], b16, tag="k_st")
                for inst in range(NI):
                    t, bb = inst // B, inst % B
                    sc = float(2.0 ** (t - 1))
                    th = float(2.0 ** t)
                    for (w_sb, bias_sb, st_tile, doff, ptag) in (
                            (wv_sb, bv4_sb, v_st, 0, "v"),
                            (wk_sb, bk4_sb, k_st, D, "k")):
                        st_ap = st_tile[:, bb * C:(bb + 1) * C]
                        ps = PSB.tile([128, C], f32, tag="psb",
                                      name=f"psb_{ptag}_{inst}")
                        for kk in range(4):
                            nc.tensor.matmul(
                                ps[:],
                                xrT_sb[:, kk * RR + inst * 128:
                                       kk * RR + (inst + 1) * 128],
                                w_sb[:, kk * C:(kk + 1) * C],
                                start=(kk == 0), stop=(kk == 3))
                        if t == 0:
                            nc.vector.scalar_tensor_tensor(
                                st_ap, ps[:], sc, bias_sb[:, t * C:(t + 1) * C],
                                ALU.mult, ALU.add)
                        else:
                            pt = SP.tile([128, C], b16, tag=f"{ptag}_pt")
                            nc.vector.scalar_tensor_tensor(
                                pt[:], ps[:], sc, bias_sb[:, t * C:(t + 1) * C],
                                ALU.mult, ALU.add)
                            nc.vector.tensor_add(st_ap, st_ap, pt[:])
                        sp = SP.tile([128, C], b16, tag=f"{ptag}_spike")
                        nc.vector.tensor_single_scalar(sp[:], st_ap, th, ALU.is_ge)
                        if t < T - 1:
                            nc.vector.scalar_tensor_tensor(
                                st_ap, st_ap, th, st_ap, ALU.is_lt, ALU.mult)
                        for hh in range(NCORES):
                            nc.gpsimd.dma_start(
                                out=vc_in[hh, inst * 128:(inst + 1) * 128,
                                          doff:doff + D],
                                in_=sp[:, hh * D:(hh + 1) * D])

                nc.gpsimd.collective_compute(
                    "AllToAll", ALU.bypass, replica_groups=rg,
                    ins=[vc_in[:]], outs=[vc_out[:]])

            # ----- stage A: q,k projection (col-shard, feature-major) -----
            with tc.tile_pool(name="psa", bufs=2, space="PSUM") as PSA, \
                 tc.tile_pool(name="sta", bufs=1) as STA:
                qk_st = STA.tile([128, RF // T], b16, tag="qk_st")
                for fi in range(RF // 512):
                    t = fi // 8
                    sc = float(2.0 ** (t - 1))
                    th = float(2.0 ** t)
                    pos = (fi % 8) * 512
                    st_ap = qk_st[:, pos:pos + 512]
                    ps = PSA.tile([128, 512], f32, tag="psa")
                    for kk in range(4):
                        xt_ = XA.tile([128, 512], b16, tag="xa")
                        nc.sync.dma_start(
                            out=xt_[:],
                            in_=xT[kk * 128:(kk + 1) * 128,
                                   fi * 512:(fi + 1) * 512])
                        nc.tensor.matmul(ps[:],
                                         wqk_sb[:, kk * 128:(kk + 1) * 128],
                                         xt_[:], start=(kk == 0), stop=(kk == 3))
                    if t == 0:
                        nc.scalar.activation(st_ap, ps[:], ACT.Identity,
                                             bias=bqk_sb[:, t:t + 1], scale=sc)
                    else:
                        pt = SP.tile([128, 512], b16, tag="qk_pt")
                        nc.scalar.activation(pt[:], ps[:], ACT.Identity,
                                             bias=bqk_sb[:, t:t + 1], scale=sc)
                        nc.vector.tensor_add(st_ap, st_ap, pt[:])
                    nc.vector.tensor_single_scalar(
                        qk_sp[:, fi * 512:(fi + 1) * 512], st_ap, th, ALU.is_ge)
                    if t < T - 1:
                        nc.vector.scalar_tensor_tensor(
                            st_ap, st_ap, th, st_ap, ALU.is_lt, ALU.mult)

            # ----- stage D: chunked retention + stage F (overlapped) -----
            with (
                tc.tile_pool(name="po", bufs=1, space="PSUM") as PO,
                tc.tile_pool(name="dg", bufs=1, space="PSUM") as DG,
                tc.tile_pool(name="kv", bufs=2, space="PSUM") as KV,
                tc.tile_pool(name="psf", bufs=2, space="PSUM") as PSF,
                tc.tile_pool(name="aw", bufs=1) as AW,
            ):
                for pi in range(8):
                    t, pj = pi // 2, pi % 2
                    insts = (t * B + 2 * pj, t * B + 2 * pj + 1)
                    sc = float(2.0 ** (t - 1))
                    po = PO.tile([128, N], f32, tag="po")
                    per_inst = []
                    for ii, inst in enumerate(insts):
                        ks = AW.tile([64, N], b16, tag=f"ks{ii}")
                        nc.sync.dma_start(
                            out=ks[:], in_=qk_sp[64:128, inst * N:(inst + 1) * N])
                        qs_ap = qk_sp[0:64, inst * N:(inst + 1) * N]
                        ql = AW.tile([64, N], b16, tag=f"ql{ii}")
                        qu = AW.tile([64, N], b16, tag=f"qu{ii}")
                        kl = AW.tile([64, N], b16, tag=f"kl{ii}")
                        ku = AW.tile([64, N], b16, tag=f"ku{ii}")
                        nc.vector.tensor_mul(ql[:], qs_ap, gsc_sb[:, 0:N])
                        nc.vector.tensor_mul(qu[:], qs_ap, gsc_sb[:, N:2 * N])
                        nc.vector.tensor_mul(kl[:], ks[:], gsc_sb[:, 2 * N:3 * N])
                        nc.vector.tensor_mul(ku[:], ks[:], gsc_sb[:, 3 * N:4 * N])
                        vkt = AW.tile([128, 8 * 128], b16, tag=f"vkt{ii}")
                        for mc in range(8):
                            nc.sync.dma_start(
                                out=vkt[:, mc * 128:(mc + 1) * 128],
                                in_=vc_out[mc, inst * 128:(inst + 1) * 128, :])
                        vfkt = AW.tile([128, 8 * 128], b16, tag=f"vfkt{ii}")
                        vbkt = AW.tile([128, 8 * 128], b16, tag=f"vbkt{ii}")
                        nc.vector.tensor_mul(vfkt[:], vkt[:], vscf_sb[:])
                        nc.vector.tensor_mul(vbkt[:], vkt[:], vscb_sb[:])
                        per_inst.append((ql, qu, kl, ku, vkt, vfkt, vbkt))

                    # --- diagonal blocks ---
                    for ii in range(2):
                        ql, qu, kl, ku, vkt, vfkt, vbkt = per_inst[ii]
                        pr = slice(64 * ii, 64 * (ii + 1))
                        for c4 in range(NC4):
                            n0 = slice(c4 * LS, c4 * LS + 128)
                            n1 = slice(c4 * LS + 128, (c4 + 1) * LS)
                            dgt = DG.tile([128, 768], f32, tag="dg",
                                          name=f"dg_{pi}_{ii}_{c4}")
                            nc.tensor.matmul(dgt[:, 0:128], kl[:, n0], ql[:, n0],
                                             start=True, stop=True)
                            nc.tensor.matmul(dgt[:, 128:256], ku[:, n0], qu[:, n0],
                                             start=True, stop=True)
                            nc.tensor.matmul(dgt[:, 256:384], kl[:, n1], ql[:, n1],
                                             start=True, stop=True)
                            nc.tensor.matmul(dgt[:, 384:512], ku[:, n1], qu[:, n1],
                                             start=True, stop=True)
                            nc.tensor.matmul(dgt[:, 512:640], kl[:, n0], ql[:, n1],
                                             start=True, stop=True)
                            nc.tensor.matmul(dgt[:, 640:768], ku[:, n1], qu[:, n0],
                                             start=True, stop=True)
                            at0 = WP.tile([128, 256], b16, tag="at0")
                            at1 = WP.tile([128, 256], b16, tag="at1")
                            nc.scalar.copy(at0[:, 0:128], dgt[:, 0:128])
                            nc.vector.copy_predicated(at0[:, 0:128], msku_sb[:],
                                                      dgt[:, 128:256])
                            nc.scalar.copy(at0[:, 128:256], dgt[:, 512:640])
                            nc.scalar.copy(at1[:, 0:128], dgt[:, 640:768])
                            nc.scalar.copy(at1[:, 128:256], dgt[:, 256:384])
                            nc.vector.copy_predicated(at1[:, 128:256], msku_sb[:],
                                                      dgt[:, 384:512])
                            m0, m1 = 2 * c4, 2 * c4 + 1
                            ncols = slice(c4 * LS, (c4 + 1) * LS)
                            nc.tensor.matmul(po[pr, ncols],
                                             vkt[:, m0 * 128:m0 * 128 + D],
                                             at0[:], start=True, stop=False)
                            nc.tensor.matmul(po[pr, ncols],
                                             vkt[:, m1 * 128:m1 * 128 + D],
                                             at1[:], start=False, stop=False)
                    # --- fwd recurrence (interleave insts per hop) ---
                    Ss, Rs_ = [], []
                    for ii in range(2):
                        Ss.append(AW.tile([64, D], b16, tag=f"S{ii}", bufs=2,
                                          name=f"S{ii}_{pi}"))
                        Rs_.append(AW.tile([64, D], b16, tag=f"R{ii}", bufs=2,
                                           name=f"R{ii}_{pi}"))
                    for c4 in range(NC4):
                        for ii in range(2):
                            ql, qu, kl, ku, vkt, vfkt, vbkt = per_inst[ii]
                            pr = slice(64 * ii, 64 * (ii + 1))
                            S = Ss[ii]
                            ncols = slice(c4 * LS, (c4 + 1) * LS)
                            if c4 > 0:
                                nc.tensor.matmul(po[pr, ncols], S[:], ql[:, ncols],
                                                 start=False, stop=(c4 == NC4 - 1))
                            kvt = KV.tile([64, D], f32, tag="kv",
                                          name=f"kvf_{pi}_{ii}_{c4}")
                            m0, m1 = 2 * c4, 2 * c4 + 1
                            nc.tensor.matmul(kvt[:],
                                             vkt[:, m0 * 128 + D:(m0 + 1) * 128],
                                             vfkt[:, m0 * 128:m0 * 128 + D],
                                             start=True, stop=False)
                            nc.tensor.matmul(kvt[:],
                                             vkt[:, m1 * 128 + D:(m1 + 1) * 128],
                                             vfkt[:, m1 * 128:m1 * 128 + D],
                                             start=False, stop=True)
                            if c4 == 0:
                                nc.scalar.copy(S[:], kvt[:])
                            elif c4 < NC4 - 1:
                                nc.vector.scalar_tensor_tensor(
                                    S[:], S[:], gLt_sb[0:64, 0:1], kvt[:],
                                    ALU.mult, ALU.add)
                    # --- bwd recurrence ---
                    for c4 in range(NC4 - 1, -1, -1):
                        for ii in range(2):
                            ql, qu, kl, ku, vkt, vfkt, vbkt = per_inst[ii]
                            pr = slice(64 * ii, 64 * (ii + 1))
                            R = Rs_[ii]
                            ncols = slice(c4 * LS, (c4 + 1) * LS)
                            if c4 < NC4 - 1:
                                nc.tensor.matmul(po[pr, ncols], R[:], qu[:, ncols],
                                                 start=False, stop=True)
                            kvt = KV.tile([64, D], f32, tag="kv",
                                          name=f"kvb_{pi}_{ii}_{c4}")
                            m0, m1 = 2 * c4, 2 * c4 + 1
                            nc.tensor.matmul(kvt[:],
                                             vkt[:, m0 * 128 + D:(m0 + 1) * 128],
                                             vbkt[:, m0 * 128:m0 * 128 + D],
                                             start=True, stop=False)
                            nc.tensor.matmul(kvt[:],
                                             vkt[:, m1 * 128 + D:(m1 + 1) * 128],
                                             vbkt[:, m1 * 128:m1 * 128 + D],
                                             start=False, stop=True)
                            if c4 == NC4 - 1:
                                nc.scalar.copy(R[:], kvt[:])
                            elif c4 > 0:
                                nc.vector.scalar_tensor_tensor(
                                    R[:], R[:], gLt_sb[0:64, 0:1], kvt[:],
                                    ALU.mult, ALU.add)

                    # --- retention LIF on po [128, N] (pair), bf16 state ---
                    st_ap = rv_st[:, pj * N:(pj + 1) * N]
                    if t == 0:
                        nc.scalar.activation(st_ap, po[:], ACT.Identity, scale=sc)
                    else:
                        rpt = SP.tile([128, N], b16, tag="r_pt")
                        nc.scalar.activation(rpt[:], po[:], ACT.Identity, scale=sc)
                        nc.vector.tensor_add(st_ap, st_ap, rpt[:])
                    rs = SP.tile([128, N], b16, tag="r_spike")
                    nc.scalar.activation(rs[:], st_ap, ACT.Sign,
                                         bias=thn_sb[:, t:t + 1])
                    if t < T - 1:
                        th = float(2.0 ** t * 0.5)
                        nc.vector.scalar_tensor_tensor(
                            st_ap, st_ap, th, st_ap, ALU.is_lt, ALU.mult)
                    rci = rc_in_a if t < 2 else rc_in_b
                    for ii, inst in enumerate(insts):
                        hoff = (inst % 8) * 128
                        for jj in range(NCORES):
                            nc.sync.dma_start(
                                out=rci[jj, :, hoff:hoff + 128],
                                in_=rs[64 * ii:64 * (ii + 1),
                                       jj * 128:(jj + 1) * 128])
                    if pi == 3:
                        nc.gpsimd.collective_compute(
                            "AllToAll", ALU.bypass, replica_groups=rg,
                            ins=[rc_in_a[:]], outs=[rc_out_a[:]])
                    if pi == 7:
                        nc.gpsimd.collective_compute(
                            "AllToAll", ALU.bypass, replica_groups=rg,
                            ins=[rc_in_b[:]], outs=[rc_out_b[:]])

                # ----- stage F: output projection + residual 1 -----
                for fc in range(4):          # fc == t (512-row slabs)
                    sc = float(2.0 ** (fc - 2))          # 0.5 * 2^{t-1}
                    th = float(2.0 ** fc)
                    rco = rc_out_a if fc < 2 else rc_out_b
                    roff = (fc % 2) * 512
                    rts = []
                    for kk in range(4):
                        rt = WP.tile([128, 512], b16, tag="wp_rhs", bufs=4)
                        nc.sync.dma_start(out=rt[0:64, :],
                                          in_=rco[2 * kk, :, roff:roff + 512])
                        nc.sync.dma_start(out=rt[64:128, :],
                                          in_=rco[2 * kk + 1, :, roff:roff + 512])
                        rts.append(rt)
                    for cc in range(4):
                        st_ap = p_st[:, cc * 512:(cc + 1) * 512]
                        ps = PSF.tile([128, 512], f32, tag="psf")
                        for kk in range(4):
                            nc.tensor.matmul(
                                ps[:],
                                wp_sb[:, kk * C + cc * 128:kk * C + (cc + 1) * 128],
                                rts[kk][:], start=(kk == 0), stop=(kk == 3))
                        if fc == 0:
                            nc.scalar.activation(
                                st_ap, ps[:], ACT.Identity,
                                bias=bpn_sb[:, cc * T + fc:cc * T + fc + 1],
                                scale=sc)
                        else:
                            pt = SP.tile([128, 512], b16, tag="p_pt")
                            nc.scalar.activation(
                                pt[:], ps[:], ACT.Identity,
                                bias=bpn_sb[:, cc * T + fc:cc * T + fc + 1],
                                scale=sc)
                            nc.vector.tensor_add(st_ap, st_ap, pt[:])
                        sp = SP.tile([128, 512], b16, tag="p_spike")
                        nc.scalar.activation(sp[:], st_ap, ACT.Sign,
                                             bias=thn_sb[:, T + fc:T + fc + 1])
                        if fc < 3:
                            nc.vector.scalar_tensor_tensor(
                                st_ap, st_ap, th, st_ap, ALU.is_lt, ALU.mult)
                        nc.vector.scalar_tensor_tensor(
                            x1p_all[:, cc * RR + fc * 512:cc * RR + (fc + 1) * 512],
                            sp[:], 0.5,
                            xp1_sb[:, cc * RR + fc * 512:cc * RR + (fc + 1) * 512],
                            ALU.mult, ALU.add)

        # ================= phase 2: MLP + residual 2 (row-shard) =================
        with tc.tile_pool(name="ph2", bufs=1) as P2:
            w1_sb = P2.tile([128, 4 * HID], b16, tag="w1_sb")
            for kk in range(4):
                nc.gpsimd.dma_start(out=w1_sb[:, kk * HID:(kk + 1) * HID],
                                    in_=w1[kk * 128:(kk + 1) * 128, :])
            w2_sb = P2.tile([128, 16 * C], b16, tag="w2_sb")
            for kk in range(16):
                nc.gpsimd.dma_start(out=w2_sb[:, kk * C:(kk + 1) * C],
                                    in_=w2[kk * 128:(kk + 1) * 128, :])
            h_st = P2.tile([128, 16 * 512], b16, tag="h_st")
            m_st = P2.tile([128, 4 * 512], b16, tag="m_st")

            with tc.tile_pool(name="psh", bufs=2, space="PSUM") as PSH, \
                 tc.tile_pool(name="psm", bufs=1, space="PSUM") as PSM:
                for rq in range(4):      # rq == t
                    sc1 = float(2.0 ** (rq - 1))
                    sc2 = float(2.0 ** (rq - 2))
                    th = float(2.0 ** rq)
                    pm = [PSM.tile([128, 512], f32, tag=f"pm{cc}",
                                   name=f"pm{cc}_{rq}") for cc in range(4)]
                    for hc in range(16):
                        st_ap = h_st[:, hc * 512:(hc + 1) * 512]
                        ph = PSH.tile([128, 512], f32, tag="psh")
                        for kk in range(4):
                            nc.tensor.matmul(
                                ph[:],
                                w1_sb[:, kk * HID + hc * 128:
                                      kk * HID + (hc + 1) * 128],
                                x1p_all[:, kk * RR + rq * 512:
                                        kk * RR + (rq + 1) * 512],
                                start=(kk == 0), stop=(kk == 3))
                        if rq == 0:
                            nc.scalar.activation(
                                st_ap, ph[:], ACT.Identity,
                                bias=b1n_sb[:, hc * T + rq:hc * T + rq + 1],
                                scale=sc1)
                        else:
                            pt = SP.tile([128, 512], b16, tag="h_pt")
                            nc.scalar.activation(
                                pt[:], ph[:], ACT.Identity,
                                bias=b1n_sb[:, hc * T + rq:hc * T + rq + 1],
                                scale=sc1)
                            nc.vector.tensor_add(st_ap, st_ap, pt[:])
                        hs = SP.tile([128, 512], b16, tag="h_spike")
                        nc.scalar.activation(hs[:], st_ap, ACT.Sign,
                                             bias=thn_sb[:, T + rq:T + rq + 1])
                        if rq < 3:
                            nc.vector.scalar_tensor_tensor(
                                st_ap, st_ap, th, st_ap, ALU.is_lt, ALU.mult)
                        for cc in range(4):
                            nc.tensor.matmul(
                                pm[cc][:],
                                w2_sb[:, hc * C + cc * 128:hc * C + (cc + 1) * 128],
                                hs[:], start=(hc == 0), stop=(hc == 15))
                    for cc in range(4):
                        st_ap = m_st[:, cc * 512:(cc + 1) * 512]
                        if rq == 0:
                            nc.scalar.activation(
                                st_ap, pm[cc][:], ACT.Identity,
                                bias=b2n_sb[:, cc * T + rq:cc * T + rq + 1],
                                scale=sc2)
                        else:
                            pt = SP.tile([128, 512], b16, tag="m_pt")
                            nc.scalar.activation(
                                pt[:], pm[cc][:], ACT.Identity,
                                bias=b2n_sb[:, cc * T + rq:cc * T + rq + 1],
                                scale=sc2)
                            nc.vector.tensor_add(st_ap, st_ap, pt[:])
                        ms = SP.tile([128, 512], b16, tag="m_spike")
                        nc.scalar.activation(ms[:], st_ap, ACT.Sign,
                                             bias=thn_sb[:, T + rq:T + rq + 1])
                        if rq < 3:
                            nc.vector.scalar_tensor_tensor(
                                st_ap, st_ap, th, st_ap, ALU.is_lt, ALU.mult)
                        ot = WP.tile([128, 512], f32, tag="ot")
                        nc.vector.scalar_tensor_tensor(
                            ot[:], ms[:], 0.5,
                            x1p_all[:, cc * RR + rq * 512:cc * RR + (rq + 1) * 512],
                            ALU.mult, ALU.add)
                        nc.sync.dma_start(
                            out=out_e[cc * 128:(cc + 1) * 128,
                                      rq * 512:(rq + 1) * 512],
                            in_=ot[:])

    nc.compile()
    return nc


def _host_prep(inputs):
    x = np.asarray(inputs["x"], np.float32)          # (T,B,N,C)
    xT_b = x.transpose(3, 0, 1, 2).reshape(C, RF).astype(bf16)

    Wq, bq_ = _fold_bn(inputs["Wq"], inputs["bq"], inputs["gq"],
                       inputs["betaq"], inputs["rmq"], inputs["rvq"])
    Wk, bk_ = _fold_bn(inputs["Wk"], inputs["bk"], inputs["gk"],
                       inputs["betak"], inputs["rmk"], inputs["rvk"])
    Wv, bv_ = _fold_bn(inputs["Wv"], inputs["bv"], inputs["gv"],
                       inputs["betav"], inputs["rmv"], inputs["rvv"])
    Wp, bp_ = _fold_bn(inputs["Wp"], inputs["bp"], inputs["gp"],
                       inputs["betap"], inputs["rmp"], inputs["rvp"])
    W1, b1_ = _fold_bn(inputs["W1"], inputs["b1"], inputs["g1"],
                       inputs["beta1"], inputs["rm1"], inputs["rv1"])
    W2, b2_ = _fold_bn(inputs["W2"], inputs["b2"], inputs["g2"],
                       inputs["beta2"], inputs["rm2"], inputs["rv2"])
    rowp = Wp.sum(axis=0).astype(np.float64)
    row1 = W1.sum(axis=0).astype(np.float64)
    row2 = W2.sum(axis=0).astype(np.float64)
    bp_n = (bp_ + 0.5 * rowp).astype(np.float32)
    b1_n = (b1_ - 0.5 * row1).astype(np.float32)
    b2_n = (b2_ + 0.5 * row2).astype(np.float32)

    tsc = np.array([2.0 ** (t - 1) for t in range(T)], np.float32)

    def pack_bias(bvec, nchunk):
        out = np.zeros((128, nchunk * T), np.float32)
        for ch in range(nchunk):
            for t in range(T):
                out[:, ch * T + t] = tsc[t] * bvec[ch * 128:(ch + 1) * 128]
        return out

    gamma = (1.0 - 2.0 ** (-5.0 - np.arange(H, dtype=np.float64)))
    jloc = np.arange(N, dtype=np.float64) % LS
    thn_m = np.zeros((128, 2 * T), np.float32)
    for t in range(T):
        thn_m[:, t] = -(2.0 ** t) * 0.5
        thn_m[:, T + t] = -(2.0 ** t)

    in_maps = []
    for cid in range(NCORES):
        h = cid
        g = gamma[h]
        gp = g ** jloc          # gamma^{n_loc}  (period LS)
        gm = g ** (-jloc)       # gamma^{-n_loc}
        gsc_m = np.zeros((64, 4 * N), np.float32)
        gsc_m[:, 0:N] = SCALE * gp[None, :]          # q_lo
        gsc_m[:, N:2 * N] = SCALE * gm[None, :]      # q_up
        gsc_m[:, 2 * N:3 * N] = gm[None, :]          # k_lo
        gsc_m[:, 3 * N:4 * N] = gp[None, :]          # k_up
        ml_, nl_ = np.meshgrid(np.arange(128), np.arange(128), indexing="ij")
        msku_m = (nl_ < ml_).astype(np.uint8)
        # vkt v-column scalings: m_loc = 128*(mc%2) + partition
        part = np.arange(128, dtype=np.float64)
        vscf_m = np.ones((128, 8 * 128), np.float32)
        vscb_m = np.ones((128, 8 * 128), np.float32)
        for mc in range(8):
            mloc = 128.0 * (mc % 2) + part
            vscf_m[:, mc * 128:mc * 128 + D] = \
                (g ** (LS - mloc))[:, None].astype(np.float32)
            vscb_m[:, mc * 128:mc * 128 + D] = \
                (g ** (LS + mloc))[:, None].astype(np.float32)

        xs = x[:, :, 128 * cid:128 * (cid + 1), :]       # (T,B,128,C)
        xrT_f = xs.transpose(3, 0, 1, 2).reshape(C, RR)

        wqk_m = np.concatenate([Wq[:, h * D:(h + 1) * D],
                                Wk[:, h * D:(h + 1) * D]], axis=1)
        bqk_m = np.zeros((128, T), np.float32)
        for t in range(T):
            bqk_m[0:64, t] = tsc[t] * bq_[h * D:(h + 1) * D]
            bqk_m[64:128, t] = tsc[t] * bk_[h * D:(h + 1) * D]

        bv4_m = np.zeros((128, 4 * C), np.float32)
        bk4_m = np.zeros((128, 4 * C), np.float32)
        for t in range(T):
            bv4_m[:, t * C:(t + 1) * C] = tsc[t] * bv_[None, :]
            bk4_m[:, t * C:(t + 1) * C] = tsc[t] * bk_[None, :]

        in_maps.append({
            "xT": xT_b,
            "xrT": xrT_f.astype(bf16),
            "xp1": (xrT_f + 1.0).astype(bf16),
            "wqk": wqk_m.astype(bf16),
            "bqk": bqk_m,
            "wv": Wv.astype(bf16),
            "bv4": bv4_m.astype(bf16),
            "wk": Wk.astype(bf16),
            "bk4": bk4_m.astype(bf16),
            "wp": Wp.astype(bf16),
            "bpn": pack_bias(bp_n, 4),
            "w1": W1.astype(bf16),
            "b1n": pack_bias(b1_n, 16),
            "w2": W2.astype(bf16),
            "b2n": pack_bias(b2_n, 4),
            "gsc": gsc_m.astype(bf16),
            "vscf": vscf_m.astype(bf16),
            "vscb": vscb_m.astype(bf16),
            "gLt": np.full((128, 1), g ** LS, np.float32),
            "thn": thn_m,
            "msku": msku_m,
        })
    return in_maps


def _install_trace_hook():
    import types
    import antenv
    if "antenv.axon_hooks" in sys.modules:
        return True
    mod = types.ModuleType("antenv.axon_hooks")
    _h = [None]
    mod.set_axon_ntff_profile_hook = lambda hk: _h.__setitem__(0, hk)
    mod.get_axon_ntff_profile_hook = lambda: _h[0]
    sys.modules["antenv.axon_hooks"] = mod
    antenv.axon_hooks = mod
    try:
        from trn_agent_boot.trn_boot import _ntff_profile_via_ctypes
        hook = _ntff_profile_via_ctypes("/opt/axon/libaxon_pjrt.so")
        mod.set_axon_ntff_profile_hook(hook)
        return hook is not None
    except Exception:
        return False


def kernel(**inputs):
    global LAST_EXEC_NS, _CACHED
    from concourse.bass_utils import run_bass_kernel_spmd

    trace = os.environ.get("BASS_KERNEL_TRACE", "0") == "1"
    if trace:
        _install_trace_hook()

    if _CACHED is None:
        _CACHED = _build_nc()
    nc = _CACHED

    in_maps = _host_prep(inputs)
    res = run_bass_kernel_spmd(nc, in_maps, core_ids=list(range(NCORES)),
                               trace=trace)
    LAST_EXEC_NS = res.exec_time_ns

    full = np.empty((T, B, N, C), np.float32)
    for cid in range(NCORES):
        oc = res.results[cid]["out"]                    # (C, RR) f32
        full[:, :, 128 * cid:128 * (cid + 1), :] = (
            oc.reshape(C, T, B, NL).transpose(1, 2, 3, 0))
    return full


# revision 11
# speedup vs baseline: 1.1553x; 1.1553x over previous
"""Trainium2 Bass kernel for nn_Block_12154757448460 (spiking retention transformer).

v3: chunked linear retention (256-wide state chunks), row-sharded k via the
v AllToAll (no PE transposes), bf16 LIF states, Sign-spike LIF on ACT for
r/p/h/m paths, engine-queue separation (sync=data DMAs, gpsimd=globals+
collectives, scalar=ACT only).

Sharding over 8 NeuronCores (1 chip):
  - q,k projections: column-sharded by head for the diagonal blocks; k is
    ALSO row-sharded through stage B so the AllToAll delivers k in [m, d]
    layout for the KV state updates (no transposes).
  - v (+k) projection, output projection, MLP: row-sharded.
  - attention: per-head chunked bidirectional retention: 128x128 masked
    diagonal blocks + 64x64 fwd/bwd decay states with 256-element hops.
  - LIF spikes on r/p/h/m paths emitted as +-1 via ACT Sign; affine fix
    folded into next layer's bias via rowsum(W). q/k/v spikes stay {0,1}.
  - BatchNorm folded into weights/biases on host; LIF in scaled-threshold
    form with bf16 states (pt terms are bf16 anyway).
"""
import os
import sys
import numpy as np
import ml_dtypes

for _p in ("/root/.axon_site/_ro/trn_rl_repo", "/opt/trn_rl_repo"):
    if os.path.isdir(_p) and _p not in sys.path:
        sys.path.append(_p)

bf16 = ml_dtypes.bfloat16

T, B, N, C = 4, 4, 1024, 512
H, D = 8, 64
HID = 4 * C
NCORES = 8
NL = N // NCORES          # 128
RR = T * B * NL           # 2048 rows per core
RF = T * B * N            # 16384 full rows
NI = T * B                # 16 attention instances
LS = 256                  # state chunk
NC4 = N // LS             # 4 state chunks
EPS = 1e-5
SCALE = D ** -0.5

LAST_EXEC_NS = None
_CACHED = None


def _fold_bn(W, bias, g, beta, rm, rv):
    ghat = (np.asarray(g, np.float64) / np.sqrt(np.asarray(rv, np.float64) + EPS))
    Wf = (np.asarray(W, np.float64) * ghat[None, :]).astype(np.float32)
    bf_ = ((np.asarray(bias, np.float64) - np.asarray(rm, np.float64)) * ghat
           + np.asarray(beta, np.float64)).astype(np.float32)
    return Wf, bf_


def _build_nc():
    import concourse.bass as bass  # noqa: F401
    import concourse.bacc as bacc
    import concourse.mybir as mybir
    from concourse import tile

    f32 = mybir.dt.float32
    b16 = mybir.dt.bfloat16
    ALU = mybir.AluOpType
    ACT = mybir.ActivationFunctionType

    nc = bacc.Bacc("TRN2", target_bir_lowering=False, debug=False,
                   num_devices=NCORES)

    # ---- external inputs (per-core values via in_maps) ----
    xT = nc.declare_dram_parameter("xT", [C, RF], b16, isOutput=False)
    xrT = nc.declare_dram_parameter("xrT", [C, RR], b16, isOutput=False)
    xp1 = nc.declare_dram_parameter("xp1", [C, RR], b16, isOutput=False)
    wqk = nc.declare_dram_parameter("wqk", [C, 128], b16, isOutput=False)
    bqk = nc.declare_dram_parameter("bqk", [128, T], f32, isOutput=False)
    wv = nc.declare_dram_parameter("wv", [C, C], b16, isOutput=False)
    bv4 = nc.declare_dram_parameter("bv4", [128, 4 * C], b16, isOutput=False)
    wk = nc.declare_dram_parameter("wk", [C, C], b16, isOutput=False)
    bk4 = nc.declare_dram_parameter("bk4", [128, 4 * C], b16, isOutput=False)
    wp = nc.declare_dram_parameter("wp", [C, C], b16, isOutput=False)
    bpn = nc.declare_dram_parameter("bpn", [128, 4 * T], f32, isOutput=False)
    w1 = nc.declare_dram_parameter("w1", [C, HID], b16, isOutput=False)
    b1n = nc.declare_dram_parameter("b1n", [128, 16 * T], f32, isOutput=False)
    w2 = nc.declare_dram_parameter("w2", [HID, C], b16, isOutput=False)
    b2n = nc.declare_dram_parameter("b2n", [128, 4 * T], f32, isOutput=False)
    gsc = nc.declare_dram_parameter("gsc", [64, 4 * N], b16, isOutput=False)
    vscf = nc.declare_dram_parameter("vscf", [128, 8 * 128], b16, isOutput=False)
    vscb = nc.declare_dram_parameter("vscb", [128, 8 * 128], b16, isOutput=False)
    gLt = nc.declare_dram_parameter("gLt", [128, 1], f32, isOutput=False)
    thn = nc.declare_dram_parameter("thn", [128, 2 * T], f32, isOutput=False)
    msku = nc.declare_dram_parameter("msku", [128, 128], mybir.dt.uint8,
                                     isOutput=False)
    out_e = nc.declare_dram_parameter("out", [C, RR], f32, isOutput=True)

    # ---- internal DRAM ----
    vc_in_a = nc.dram_tensor("vc_in_a", [NCORES, RR // 2, 2 * D], b16)
    vc_out_a = nc.dram_tensor("vc_out_a", [NCORES, RR // 2, 2 * D], b16)
    vc_in_b = nc.dram_tensor("vc_in_b", [NCORES, RR // 2, 2 * D], b16)
    vc_out_b = nc.dram_tensor("vc_out_b", [NCORES, RR // 2, 2 * D], b16)
    rc_in_a = nc.dram_tensor("rc_in_a", [NCORES, D, RR // 2], b16)
    rc_out_a = nc.dram_tensor("rc_out_a", [NCORES, D, RR // 2], b16)
    rc_in_b = nc.dram_tensor("rc_in_b", [NCORES, D, RR // 2], b16)
    rc_out_b = nc.dram_tensor("rc_out_b", [NCORES, D, RR // 2], b16)

    rg = [list(range(NCORES))]

    with tile.TileContext(nc) as tc:
      with (
          tc.tile_pool(name="glob", bufs=1) as GP,
          tc.tile_pool(name="work", bufs=3) as WP,
          tc.tile_pool(name="spike", bufs=2) as SP,
          tc.tile_pool(name="xa", bufs=6) as XA,
      ):
        # ---------- stage-B-critical globals first (sync queue) ----------
        xrT_sb = GP.tile([128, 4 * RR], b16, tag="xrT_sb")
        for kk in range(4):
            nc.sync.dma_start(out=xrT_sb[:, kk * RR:(kk + 1) * RR],
                              in_=xrT[kk * 128:(kk + 1) * 128, :])
        wv_sb = GP.tile([128, 4 * C], b16, tag="wv_sb")
        for kk in range(4):
            nc.sync.dma_start(out=wv_sb[:, kk * C:(kk + 1) * C],
                              in_=wv[kk * 128:(kk + 1) * 128, :])
        wk_sb = GP.tile([128, 4 * C], b16, tag="wk_sb")
        for kk in range(4):
            nc.sync.dma_start(out=wk_sb[:, kk * C:(kk + 1) * C],
                              in_=wk[kk * 128:(kk + 1) * 128, :])
        bv4_sb = GP.tile([128, 4 * C], b16, tag="bv4_sb")
        nc.sync.dma_start(out=bv4_sb[:], in_=bv4[:])
        bk4_sb = GP.tile([128, 4 * C], b16, tag="bk4_sb")
        nc.sync.dma_start(out=bk4_sb[:], in_=bk4[:])
        # ---------- remaining globals (gpsimd queue) ----------
        wqk_sb = GP.tile([128, 4 * 128], b16, tag="wqk_sb")
        for kk in range(4):
            nc.gpsimd.dma_start(out=wqk_sb[:, kk * 128:(kk + 1) * 128],
                                in_=wqk[kk * 128:(kk + 1) * 128, :])
        bqk_sb = GP.tile([128, T], f32, tag="bqk_sb")
        nc.gpsimd.dma_start(out=bqk_sb[:], in_=bqk[:])
        gsc_sb = GP.tile([64, 4 * N], b16, tag="gsc_sb")
        nc.gpsimd.dma_start(out=gsc_sb[:], in_=gsc[:])
        vscf_sb = GP.tile([128, 8 * 128], b16, tag="vscf_sb")
        nc.gpsimd.dma_start(out=vscf_sb[:], in_=vscf[:])
        vscb_sb = GP.tile([128, 8 * 128], b16, tag="vscb_sb")
        nc.gpsimd.dma_start(out=vscb_sb[:], in_=vscb[:])
        gLt_sb = GP.tile([128, 1], f32, tag="gLt_sb")
        nc.gpsimd.dma_start(out=gLt_sb[:], in_=gLt[:])
        thn_sb = GP.tile([128, 2 * T], f32, tag="thn_sb")
        nc.gpsimd.dma_start(out=thn_sb[:], in_=thn[:])
        msku_sb = GP.tile([128, 128], mybir.dt.uint8, tag="msku_sb")
        nc.gpsimd.dma_start(out=msku_sb[:], in_=msku[:])
        xp1_sb = GP.tile([128, 4 * RR], b16, tag="xp1_sb")
        for kk in range(4):
            nc.gpsimd.dma_start(out=xp1_sb[:, kk * RR:(kk + 1) * RR],
                                in_=xp1[kk * 128:(kk + 1) * 128, :])
        wp_sb = GP.tile([128, 4 * C], b16, tag="wp_sb")
        for kk in range(4):
            nc.gpsimd.dma_start(out=wp_sb[:, kk * C:(kk + 1) * C],
                                in_=wp[kk * 128:(kk + 1) * 128, :])
        bpn_sb = GP.tile([128, 4 * T], f32, tag="bpn_sb")
        nc.gpsimd.dma_start(out=bpn_sb[:], in_=bpn[:])
        b1n_sb = GP.tile([128, 16 * T], f32, tag="b1n_sb")
        nc.gpsimd.dma_start(out=b1n_sb[:], in_=b1n[:])
        b2n_sb = GP.tile([128, 4 * T], f32, tag="b2n_sb")
        nc.gpsimd.dma_start(out=b2n_sb[:], in_=b2n[:])
        # cross-phase result tiles
        x1p_all = GP.tile([128, 4 * RR], b16, tag="x1p_all")
        p_st = GP.tile([128, 4 * 512], b16, tag="p_st")

        # ================= phase 1 =================
        with tc.tile_pool(name="ph1", bufs=1) as P1:
            qk_sp = P1.tile([128, RF], b16, tag="qk_sp")   # [q_h; k_h] spikes
            rv_st = P1.tile([128, 2 * N], b16, tag="rv_st")  # 2 pair slots

            # ----- stage B: v,k projections (row-shard, row-major) -----
            with tc.tile_pool(name="psb", bufs=2, space="PSUM") as PSB, \
                 tc.tile_pool(name="stb", bufs=1) as STB:
                v_st = STB.tile([128, B * C], b16, tag="v_st")
                k_st = STB.tile([128, B * C], b16, tag="k_st")
                for inst in range(NI):
                    t, bb = inst // B, inst % B
                    sc = float(2.0 ** (t - 1))
                    th = float(2.0 ** t)
                    for (w_sb, bias_sb, st_tile, doff, ptag) in (
                            (wv_sb, bv4_sb, v_st, 0, "v"),
                            (wk_sb, bk4_sb, k_st, D, "k")):
                        st_ap = st_tile[:, bb * C:(bb + 1) * C]
                        ps = PSB.tile([128, C], f32, tag="psb",
                                      name=f"psb_{ptag}_{inst}")
                        for kk in range(4):
                            nc.tensor.matmul(
                                ps[:],
                                xrT_sb[:, kk * RR + inst * 128:
                                       kk * RR + (inst + 1) * 128],
                                w_sb[:, kk * C:(kk + 1) * C],
                                start=(kk == 0), stop=(kk == 3))
                        if t == 0:
                            nc.vector.scalar_tensor_tensor(
                                st_ap, ps[:], sc, bias_sb[:, t * C:(t + 1) * C],
                                ALU.mult, ALU.add)
                        else:
                            pt = SP.tile([128, C], b16, tag=f"{ptag}_pt")
                            nc.vector.scalar_tensor_tensor(
                                pt[:], ps[:], sc, bias_sb[:, t * C:(t + 1) * C],
                                ALU.mult, ALU.add)
                            nc.vector.tensor_add(st_ap, st_ap, pt[:])
                        sp = SP.tile([128, C], b16, tag=f"{ptag}_spike")
                        nc.vector.tensor_single_scalar(sp[:], st_ap, th, ALU.is_ge)
                        if t < T - 1:
                            nc.vector.scalar_tensor_tensor(
                                st_ap, st_ap, th, st_ap, ALU.is_lt, ALU.mult)
                        vci = vc_in_a if inst < 8 else vc_in_b
                        il = inst % 8
                        nc.sync.dma_start(
                            out=vci[:, il * 128:(il + 1) * 128,
                                    doff:doff + D].transpose([1, 0, 2]),
                            in_=sp[:])
                    if inst == 7:
                        nc.gpsimd.collective_compute(
                            "AllToAll", ALU.bypass, replica_groups=rg,
                            ins=[vc_in_a[:]], outs=[vc_out_a[:]])
                    if inst == 15:
                        nc.gpsimd.collective_compute(
                            "AllToAll", ALU.bypass, replica_groups=rg,
                            ins=[vc_in_b[:]], outs=[vc_out_b[:]])

            # ----- stage A: q,k projection (col-shard, feature-major) -----
            with tc.tile_pool(name="psa", bufs=2, space="PSUM") as PSA, \
                 tc.tile_pool(name="sta", bufs=1) as STA:
                qk_st = STA.tile([128, RF // T], b16, tag="qk_st")
                for fi in range(RF // 512):
                    t = fi // 8
                    sc = float(2.0 ** (t - 1))
                    th = float(2.0 ** t)
                    pos = (fi % 8) * 512
                    st_ap = qk_st[:, pos:pos + 512]
                    ps = PSA.tile([128, 512], f32, tag="psa")
                    for kk in range(4):
                        xt_ = XA.tile([128, 512], b16, tag="xa")
                        nc.sync.dma_start(
                            out=xt_[:],
                            in_=xT[kk * 128:(kk + 1) * 128,
                                   fi * 512:(fi + 1) * 512])
                        nc.tensor.matmul(ps[:],
                                         wqk_sb[:, kk * 128:(kk + 1) * 128],
                                         xt_[:], start=(kk == 0), stop=(kk == 3))
                    if t == 0:
                        nc.scalar.activation(st_ap, ps[:], ACT.Identity,
                                             bias=bqk_sb[:, t:t + 1], scale=sc)
                    else:
                        pt = SP.tile([128, 512], b16, tag="qk_pt")
                        nc.scalar.activation(pt[:], ps[:], ACT.Identity,
                                             bias=bqk_sb[:, t:t + 1], scale=sc)
                        nc.vector.tensor_add(st_ap, st_ap, pt[:])
                    nc.vector.tensor_single_scalar(
                        qk_sp[:, fi * 512:(fi + 1) * 512], st_ap, th, ALU.is_ge)
                    if t < T - 1:
                        nc.vector.scalar_tensor_tensor(
                            st_ap, st_ap, th, st_ap, ALU.is_lt, ALU.mult)

            # ----- stage D: chunked retention + stage F (overlapped) -----
            with (
                tc.tile_pool(name="po", bufs=1, space="PSUM") as PO,
                tc.tile_pool(name="dg", bufs=1, space="PSUM") as DG,
                tc.tile_pool(name="kv", bufs=2, space="PSUM") as KV,
                tc.tile_pool(name="psf", bufs=2, space="PSUM") as PSF,
                tc.tile_pool(name="aw", bufs=1) as AW,
            ):
                for pi in range(8):
                    t, pj = pi // 2, pi % 2
                    insts = (t * B + 2 * pj, t * B + 2 * pj + 1)
                    sc = float(2.0 ** (t - 1))
                    po = PO.tile([128, N], f32, tag="po")
                    per_inst = []
                    for ii, inst in enumerate(insts):
                        ks = AW.tile([64, N], b16, tag=f"ks{ii}")
                        nc.sync.dma_start(
                            out=ks[:], in_=qk_sp[64:128, inst * N:(inst + 1) * N])
                        qs_ap = qk_sp[0:64, inst * N:(inst + 1) * N]
                        ql = AW.tile([64, N], b16, tag=f"ql{ii}")
                        qu = AW.tile([64, N], b16, tag=f"qu{ii}")
                        kl = AW.tile([64, N], b16, tag=f"kl{ii}")
                        ku = AW.tile([64, N], b16, tag=f"ku{ii}")
                        nc.vector.tensor_mul(ql[:], qs_ap, gsc_sb[:, 0:N])
                        nc.vector.tensor_mul(qu[:], qs_ap, gsc_sb[:, N:2 * N])
                        nc.vector.tensor_mul(kl[:], ks[:], gsc_sb[:, 2 * N:3 * N])
                        nc.vector.tensor_mul(ku[:], ks[:], gsc_sb[:, 3 * N:4 * N])
                        vkt = AW.tile([128, 8 * 128], b16, tag=f"vkt{ii}")
                        vco = vc_out_a if inst < 8 else vc_out_b
                        il = inst % 8
                        nc.sync.dma_start(
                            out=vkt[:],
                            in_=vco[:, il * 128:(il + 1) * 128,
                                    :].transpose([1, 0, 2]))
                        vfkt = AW.tile([128, 8 * 128], b16, tag=f"vfkt{ii}")
                        vbkt = AW.tile([128, 8 * 128], b16, tag=f"vbkt{ii}")
                        nc.vector.tensor_mul(vfkt[:], vkt[:], vscf_sb[:])
                        nc.vector.tensor_mul(vbkt[:], vkt[:], vscb_sb[:])
                        per_inst.append((ql, qu, kl, ku, vkt, vfkt, vbkt))

                    # --- diagonal blocks ---
                    for ii in range(2):
                        ql, qu, kl, ku, vkt, vfkt, vbkt = per_inst[ii]
                        pr = slice(64 * ii, 64 * (ii + 1))
                        for c4 in range(NC4):
                            n0 = slice(c4 * LS, c4 * LS + 128)
                            n1 = slice(c4 * LS + 128, (c4 + 1) * LS)
                            dgt = DG.tile([128, 768], f32, tag="dg",
                                          name=f"dg_{pi}_{ii}_{c4}")
                            nc.tensor.matmul(dgt[:, 0:128], kl[:, n0], ql[:, n0],
                                             start=True, stop=True)
                            nc.tensor.matmul(dgt[:, 128:256], ku[:, n0], qu[:, n0],
                                             start=True, stop=True)
                            nc.tensor.matmul(dgt[:, 256:384], kl[:, n1], ql[:, n1],
                                             start=True, stop=True)
                            nc.tensor.matmul(dgt[:, 384:512], ku[:, n1], qu[:, n1],
                                             start=True, stop=True)
                            nc.tensor.matmul(dgt[:, 512:640], kl[:, n0], ql[:, n1],
                                             start=True, stop=True)
                            nc.tensor.matmul(dgt[:, 640:768], ku[:, n1], qu[:, n0],
                                             start=True, stop=True)
                            at0 = WP.tile([128, 256], b16, tag="at0")
                            at1 = WP.tile([128, 256], b16, tag="at1")
                            nc.scalar.copy(at0[:, 0:128], dgt[:, 0:128])
                            nc.vector.copy_predicated(at0[:, 0:128], msku_sb[:],
                                                      dgt[:, 128:256])
                            nc.scalar.copy(at0[:, 128:256], dgt[:, 512:640])
                            nc.scalar.copy(at1[:, 0:128], dgt[:, 640:768])
                            nc.scalar.copy(at1[:, 128:256], dgt[:, 256:384])
                            nc.vector.copy_predicated(at1[:, 128:256], msku_sb[:],
                                                      dgt[:, 384:512])
                            m0, m1 = 2 * c4, 2 * c4 + 1
                            ncols = slice(c4 * LS, (c4 + 1) * LS)
                            nc.tensor.matmul(po[pr, ncols],
                                             vkt[:, m0 * 128:m0 * 128 + D],
                                             at0[:], start=True, stop=False)
                            nc.tensor.matmul(po[pr, ncols],
                                             vkt[:, m1 * 128:m1 * 128 + D],
                                             at1[:], start=False, stop=False)
                    # --- fwd recurrence (interleave insts per hop) ---
                    Ss, Rs_ = [], []
                    for ii in range(2):
                        Ss.append(AW.tile([64, D], b16, tag=f"S{ii}", bufs=2,
                                          name=f"S{ii}_{pi}"))
                        Rs_.append(AW.tile([64, D], b16, tag=f"R{ii}", bufs=2,
                                           name=f"R{ii}_{pi}"))
                    for c4 in range(NC4):
                        for ii in range(2):
                            ql, qu, kl, ku, vkt, vfkt, vbkt = per_inst[ii]
                            pr = slice(64 * ii, 64 * (ii + 1))
                            S = Ss[ii]
                            ncols = slice(c4 * LS, (c4 + 1) * LS)
                            if c4 > 0:
                                nc.tensor.matmul(po[pr, ncols], S[:], ql[:, ncols],
                                                 start=False, stop=(c4 == NC4 - 1))
                            kvt = KV.tile([64, D], f32, tag="kv",
                                          name=f"kvf_{pi}_{ii}_{c4}")
                            m0, m1 = 2 * c4, 2 * c4 + 1
                            nc.tensor.matmul(kvt[:],
                                             vkt[:, m0 * 128 + D:(m0 + 1) * 128],
                                             vfkt[:, m0 * 128:m0 * 128 + D],
                                             start=True, stop=False)
                            nc.tensor.matmul(kvt[:],
                                             vkt[:, m1 * 128 + D:(m1 + 1) * 128],
                                             vfkt[:, m1 * 128:m1 * 128 + D],
                                             start=False, stop=True)
                            if c4 == 0:
                                nc.scalar.copy(S[:], kvt[:])
                            elif c4 < NC4 - 1:
                                nc.vector.scalar_tensor_tensor(
                                    S[:], S[:], gLt_sb[0:64, 0:1], kvt[:],
                                    ALU.mult, ALU.add)
                    # --- bwd recurrence ---
                    for c4 in range(NC4 - 1, -1, -1):
                        for ii in range(2):
                            ql, qu, kl, ku, vkt, vfkt, vbkt = per_inst[ii]
                            pr = slice(64 * ii, 64 * (ii + 1))
                            R = Rs_[ii]
                            ncols = slice(c4 * LS, (c4 + 1) * LS)
                            if c4 < NC4 - 1:
                                nc.tensor.matmul(po[pr, ncols], R[:], qu[:, ncols],
                                                 start=False, stop=True)
                            kvt = KV.tile([64, D], f32, tag="kv",
                                          name=f"kvb_{pi}_{ii}_{c4}")
                            m0, m1 = 2 * c4, 2 * c4 + 1
                            nc.tensor.matmul(kvt[:],
                                             vkt[:, m0 * 128 + D:(m0 + 1) * 128],
                                             vbkt[:, m0 * 128:m0 * 128 + D],
                                             start=True, stop=False)
                            nc.tensor.matmul(kvt[:],
                                             vkt[:, m1 * 128 + D:(m1 + 1) * 128],
                                             vbkt[:, m1 * 128:m1 * 128 + D],
                                             start=False, stop=True)
                            if c4 == NC4 - 1:
                                nc.scalar.copy(R[:], kvt[:])
                            elif c4 > 0:
                                nc.vector.scalar_tensor_tensor(
                                    R[:], R[:], gLt_sb[0:64, 0:1], kvt[:],
                                    ALU.mult, ALU.add)

                    # --- retention LIF on po [128, N] (pair), bf16 state ---
                    st_ap = rv_st[:, pj * N:(pj + 1) * N]
                    if t == 0:
                        nc.scalar.activation(st_ap, po[:], ACT.Identity, scale=sc)
                    else:
                        rpt = SP.tile([128, N], b16, tag="r_pt")
                        nc.scalar.activation(rpt[:], po[:], ACT.Identity, scale=sc)
                        nc.vector.tensor_add(st_ap, st_ap, rpt[:])
                    rs = SP.tile([128, N], b16, tag="r_spike")
                    nc.scalar.activation(rs[:], st_ap, ACT.Sign,
                                         bias=thn_sb[:, t:t + 1])
                    if t < T - 1:
                        th = float(2.0 ** t * 0.5)
                        nc.vector.scalar_tensor_tensor(
                            st_ap, st_ap, th, st_ap, ALU.is_lt, ALU.mult)
                    rci = rc_in_a if t < 2 else rc_in_b
                    for ii, inst in enumerate(insts):
                        hoff = (inst % 8) * 128
                        nc.sync.dma_start(
                            out=rci[:, :, hoff:hoff + 128].transpose([1, 0, 2]),
                            in_=rs[64 * ii:64 * (ii + 1), :])
                    if pi == 3:
                        nc.gpsimd.collective_compute(
                            "AllToAll", ALU.bypass, replica_groups=rg,
                            ins=[rc_in_a[:]], outs=[rc_out_a[:]])
                    if pi == 7:
                        nc.gpsimd.collective_compute(
                            "AllToAll", ALU.bypass, replica_groups=rg,
                            ins=[rc_in_b[:]], outs=[rc_out_b[:]])

                # ----- stage F: output projection + residual 1 -----
                for fc in range(4):          # fc == t (512-row slabs)
                    sc = float(2.0 ** (fc - 2))          # 0.5 * 2^{t-1}
                    th = float(2.0 ** fc)
                    rco = rc_out_a if fc < 2 else rc_out_b
                    roff = (fc % 2) * 512
                    rts = []
                    for kk in range(4):
                        rt = WP.tile([128, 512], b16, tag="wp_rhs", bufs=4)
                        nc.sync.dma_start(
                            out=rt[:],
                            in_=rco[2 * kk:2 * kk + 2, :, roff:roff + 512])
                        rts.append(rt)
                    for cc in range(4):
                        st_ap = p_st[:, cc * 512:(cc + 1) * 512]
                        ps = PSF.tile([128, 512], f32, tag="psf")
                        for kk in range(4):
                            nc.tensor.matmul(
                                ps[:],
                                wp_sb[:, kk * C + cc * 128:kk * C + (cc + 1) * 128],
                                rts[kk][:], start=(kk == 0), stop=(kk == 3))
                        if fc == 0:
                            nc.scalar.activation(
                                st_ap, ps[:], ACT.Identity,
                                bias=bpn_sb[:, cc * T + fc:cc * T + fc + 1],
                                scale=sc)
                        else:
                            pt = SP.tile([128, 512], b16, tag="p_pt")
                            nc.scalar.activation(
                                pt[:], ps[:], ACT.Identity,
                                bias=bpn_sb[:, cc * T + fc:cc * T + fc + 1],
                                scale=sc)
                            nc.vector.tensor_add(st_ap, st_ap, pt[:])
                        sp = SP.tile([128, 512], b16, tag="p_spike")
                        nc.scalar.activation(sp[:], st_ap, ACT.Sign,
                                             bias=thn_sb[:, T + fc:T + fc + 1])
                        if fc < 3:
                            nc.vector.scalar_tensor_tensor(
                                st_ap, st_ap, th, st_ap, ALU.is_lt, ALU.mult)
                        nc.vector.scalar_tensor_tensor(
                            x1p_all[:, cc * RR + fc * 512:cc * RR + (fc + 1) * 512],
                            sp[:], 0.5,
                            xp1_sb[:, cc * RR + fc * 512:cc * RR + (fc + 1) * 512],
                            ALU.mult, ALU.add)

        # ================= phase 2: MLP + residual 2 (row-shard) =================
        with tc.tile_pool(name="ph2", bufs=1) as P2:
            w1_sb = P2.tile([128, 4 * HID], b16, tag="w1_sb")
            for kk in range(4):
                nc.gpsimd.dma_start(out=w1_sb[:, kk * HID:(kk + 1) * HID],
                                    in_=w1[kk * 128:(kk + 1) * 128, :])
            w2_sb = P2.tile([128, 16 * C], b16, tag="w2_sb")
            for kk in range(16):
                nc.gpsimd.dma_start(out=w2_sb[:, kk * C:(kk + 1) * C],
                                    in_=w2[kk * 128:(kk + 1) * 128, :])
            h_st = P2.tile([128, 16 * 512], b16, tag="h_st")
            m_st = P2.tile([128, 4 * 512], b16, tag="m_st")

            with tc.tile_pool(name="psh", bufs=2, space="PSUM") as PSH, \
                 tc.tile_pool(name="psm", bufs=1, space="PSUM") as PSM:
                for rq in range(4):      # rq == t
                    sc1 = float(2.0 ** (rq - 1))
                    sc2 = float(2.0 ** (rq - 2))
                    th = float(2.0 ** rq)
                    pm = [PSM.tile([128, 512], f32, tag=f"pm{cc}",
                                   name=f"pm{cc}_{rq}") for cc in range(4)]
                    for hc in range(16):
                        st_ap = h_st[:, hc * 512:(hc + 1) * 512]
                        ph = PSH.tile([128, 512], f32, tag="psh")
                        for kk in range(4):
                            nc.tensor.matmul(
                                ph[:],
                                w1_sb[:, kk * HID + hc * 128:
                                      kk * HID + (hc + 1) * 128],
                                x1p_all[:, kk * RR + rq * 512:
                                        kk * RR + (rq + 1) * 512],
                                start=(kk == 0), stop=(kk == 3))
                        if rq == 0:
                            nc.scalar.activation(
                                st_ap, ph[:], ACT.Identity,
                                bias=b1n_sb[:, hc * T + rq:hc * T + rq + 1],
                                scale=sc1)
                        else:
                            pt = SP.tile([128, 512], b16, tag="h_pt")
                            nc.scalar.activation(
                                pt[:], ph[:], ACT.Identity,
                                bias=b1n_sb[:, hc * T + rq:hc * T + rq + 1],
                                scale=sc1)
                            nc.vector.tensor_add(st_ap, st_ap, pt[:])
                        hs = SP.tile([128, 512], b16, tag="h_spike")
                        nc.scalar.activation(hs[:], st_ap, ACT.Sign,
                                             bias=thn_sb[:, T + rq:T + rq + 1])
                        if rq < 3:
                            nc.vector.scalar_tensor_tensor(
                                st_ap, st_ap, th, st_ap, ALU.is_lt, ALU.mult)
                        for cc in range(4):
                            nc.tensor.matmul(
                                pm[cc][:],
                                w2_sb[:, hc * C + cc * 128:hc * C + (cc + 1) * 128],
                                hs[:], start=(hc == 0), stop=(hc == 15))
                    for cc in range(4):
                        st_ap = m_st[:, cc * 512:(cc + 1) * 512]
                        if rq == 0:
                            nc.scalar.activation(
                                st_ap, pm[cc][:], ACT.Identity,
                                bias=b2n_sb[:, cc * T + rq:cc * T + rq + 1],
                                scale=sc2)
                        else:
                            pt = SP.tile([128, 512], b16, tag="m_pt")
                            nc.scalar.activation(
                                pt[:], pm[cc][:], ACT.Identity,
                                bias=b2n_sb[:, cc * T + rq:cc * T + rq + 1],
                                scale=sc2)
                            nc.vector.tensor_add(st_ap, st_ap, pt[:])
                        ms = SP.tile([128, 512], b16, tag="m_spike")
                        nc.scalar.activation(ms[:], st_ap, ACT.Sign,
                                             bias=thn_sb[:, T + rq:T + rq + 1])
                        if rq < 3:
                            nc.vector.scalar_tensor_tensor(
                                st_ap, st_ap, th, st_ap, ALU.is_lt, ALU.mult)
                        ot = WP.tile([128, 512], f32, tag="ot")
                        nc.vector.scalar_tensor_tensor(
                            ot[:], ms[:], 0.5,
                            x1p_all[:, cc * RR + rq * 512:cc * RR + (rq + 1) * 512],
                            ALU.mult, ALU.add)
                        nc.sync.dma_start(
                            out=out_e[cc * 128:(cc + 1) * 128,
                                      rq * 512:(rq + 1) * 512],
                            in_=ot[:])

    nc.compile()
    return nc


def _host_prep(inputs):
    x = np.asarray(inputs["x"], np.float32)          # (T,B,N,C)
    xT_b = x.transpose(3, 0, 1, 2).reshape(C, RF).astype(bf16)

    Wq, bq_ = _fold_bn(inputs["Wq"], inputs["bq"], inputs["gq"],
                       inputs["betaq"], inputs["rmq"], inputs["rvq"])
    Wk, bk_ = _fold_bn(inputs["Wk"], inputs["bk"], inputs["gk"],
                       inputs["betak"], inputs["rmk"], inputs["rvk"])
    Wv, bv_ = _fold_bn(inputs["Wv"], inputs["bv"], inputs["gv"],
                       inputs["betav"], inputs["rmv"], inputs["rvv"])
    Wp, bp_ = _fold_bn(inputs["Wp"], inputs["bp"], inputs["gp"],
                       inputs["betap"], inputs["rmp"], inputs["rvp"])
    W1, b1_ = _fold_bn(inputs["W1"], inputs["b1"], inputs["g1"],
                       inputs["beta1"], inputs["rm1"], inputs["rv1"])
    W2, b2_ = _fold_bn(inputs["W2"], inputs["b2"], inputs["g2"],
                       inputs["beta2"], inputs["rm2"], inputs["rv2"])
    rowp = Wp.sum(axis=0).astype(np.float64)
    row1 = W1.sum(axis=0).astype(np.float64)
    row2 = W2.sum(axis=0).astype(np.float64)
    bp_n = (bp_ + 0.5 * rowp).astype(np.float32)
    b1_n = (b1_ - 0.5 * row1).astype(np.float32)
    b2_n = (b2_ + 0.5 * row2).astype(np.float32)

    tsc = np.array([2.0 ** (t - 1) for t in range(T)], np.float32)

    def pack_bias(bvec, nchunk):
        out = np.zeros((128, nchunk * T), np.float32)
        for ch in range(nchunk):
            for t in range(T):
                out[:, ch * T + t] = tsc[t] * bvec[ch * 128:(ch + 1) * 128]
        return out

    gamma = (1.0 - 2.0 ** (-5.0 - np.arange(H, dtype=np.float64)))
    jloc = np.arange(N, dtype=np.float64) % LS
    thn_m = np.zeros((128, 2 * T), np.float32)
    for t in range(T):
        thn_m[:, t] = -(2.0 ** t) * 0.5
        thn_m[:, T + t] = -(2.0 ** t)

    in_maps = []
    for cid in range(NCORES):
        h = cid
        g = gamma[h]
        gp = g ** jloc          # gamma^{n_loc}  (period LS)
        gm = g ** (-jloc)       # gamma^{-n_loc}
        gsc_m = np.zeros((64, 4 * N), np.float32)
        gsc_m[:, 0:N] = SCALE * gp[None, :]          # q_lo
        gsc_m[:, N:2 * N] = SCALE * gm[None, :]      # q_up
        gsc_m[:, 2 * N:3 * N] = gm[None, :]          # k_lo
        gsc_m[:, 3 * N:4 * N] = gp[None, :]          # k_up
        ml_, nl_ = np.meshgrid(np.arange(128), np.arange(128), indexing="ij")
        msku_m = (nl_ < ml_).astype(np.uint8)
        # vkt v-column scalings: m_loc = 128*(mc%2) + partition
        part = np.arange(128, dtype=np.float64)
        vscf_m = np.ones((128, 8 * 128), np.float32)
        vscb_m = np.ones((128, 8 * 128), np.float32)
        for mc in range(8):
            mloc = 128.0 * (mc % 2) + part
            vscf_m[:, mc * 128:mc * 128 + D] = \
                (g ** (LS - mloc))[:, None].astype(np.float32)
            vscb_m[:, mc * 128:mc * 128 + D] = \
                (g ** (LS + mloc))[:, None].astype(np.float32)

        xs = x[:, :, 128 * cid:128 * (cid + 1), :]       # (T,B,128,C)
        xrT_f = xs.transpose(3, 0, 1, 2).reshape(C, RR)

        wqk_m = np.concatenate([Wq[:, h * D:(h + 1) * D],
                                Wk[:, h * D:(h + 1) * D]], axis=1)
        bqk_m = np.zeros((128, T), np.float32)
        for t in range(T):
            bqk_m[0:64, t] = tsc[t] * bq_[h * D:(h + 1) * D]
            bqk_m[64:128, t] = tsc[t] * bk_[h * D:(h + 1) * D]

        bv4_m = np.zeros((128, 4 * C), np.float32)
        bk4_m = np.zeros((128, 4 * C), np.float32)
        for t in range(T):
            bv4_m[:, t * C:(t + 1) * C] = tsc[t] * bv_[None, :]
            bk4_m[:, t * C:(t + 1) * C] = tsc[t] * bk_[None, :]

        in_maps.append({
            "xT": xT_b,
            "xrT": xrT_f.astype(bf16),
            "xp1": (xrT_f + 1.0).astype(bf16),
            "wqk": wqk_m.astype(bf16),
            "bqk": bqk_m,
            "wv": Wv.astype(bf16),
            "bv4": bv4_m.astype(bf16),
            "wk": Wk.astype(bf16),
            "bk4": bk4_m.astype(bf16),
            "wp": Wp.astype(bf16),
            "bpn": pack_bias(bp_n, 4),
            "w1": W1.astype(bf16),
            "b1n": pack_bias(b1_n, 16),
            "w2": W2.astype(bf16),
            "b2n": pack_bias(b2_n, 4),
            "gsc": gsc_m.astype(bf16),
            "vscf": vscf_m.astype(bf16),
            "vscb": vscb_m.astype(bf16),
            "gLt": np.full((128, 1), g ** LS, np.float32),
            "thn": thn_m,
            "msku": msku_m,
        })
    return in_maps


def _install_trace_hook():
    import types
    import antenv
    if "antenv.axon_hooks" in sys.modules:
        return True
    mod = types.ModuleType("antenv.axon_hooks")
    _h = [None]
    mod.set_axon_ntff_profile_hook = lambda hk: _h.__setitem__(0, hk)
    mod.get_axon_ntff_profile_hook = lambda: _h[0]
    sys.modules["antenv.axon_hooks"] = mod
    antenv.axon_hooks = mod
    try:
        from trn_agent_boot.trn_boot import _ntff_profile_via_ctypes
        hook = _ntff_profile_via_ctypes("/opt/axon/libaxon_pjrt.so")
        mod.set_axon_ntff_profile_hook(hook)
        return hook is not None
    except Exception:
        return False


def kernel(**inputs):
    global LAST_EXEC_NS, _CACHED
    from concourse.bass_utils import run_bass_kernel_spmd

    trace = os.environ.get("BASS_KERNEL_TRACE", "0") == "1"
    if trace:
        _install_trace_hook()

    if _CACHED is None:
        _CACHED = _build_nc()
    nc = _CACHED

    in_maps = _host_prep(inputs)
    res = run_bass_kernel_spmd(nc, in_maps, core_ids=list(range(NCORES)),
                               trace=trace)
    LAST_EXEC_NS = res.exec_time_ns

    full = np.empty((T, B, N, C), np.float32)
    for cid in range(NCORES):
        oc = res.results[cid]["out"]                    # (C, RR) f32
        full[:, :, 128 * cid:128 * (cid + 1), :] = (
            oc.reshape(C, T, B, NL).transpose(1, 2, 3, 0))
    return full
